# revision 29
# baseline (speedup 1.0000x reference)
"""Cross-temporal attention Trainium2 (Bass/Tile) kernel.

Problem: two streams x1, x2 of shape [B=4, C=256, H=64, W=64]; tokens are the
H*W=4096 spatial positions. Per batch b and stream s:
    q_s = t_s @ Wq.T + bq ; k_s = t_s @ Wk.T + bk ; v_s = t_s @ Wv.T + bv
    out_s = softmax(q_s @ k_{3-s}.T) @ v_s            (no 1/sqrt(d) scaling)

Sharding: 8 NeuronCores, one (batch, stream) unit per core (4 batches x 2
streams). Fully SPMD — the same program runs on every core, only the input
bindings differ. No collectives.

Key algebraic restructure (v7): the Q/K projections fold into ONE matrix.
    softmax_m(q_n . k_m) = softmax_m(x1_n^T Wq^T Wk x2_m + bq . Wk x2_m)
(all per-query-constant terms cancel inside the softmax over keys m). With
M := Wq^T Wk and r := Wk^T bq, define T1' := M^T X1 + r (a single projection
of x1); then softmax logits^T = X2^T T1' exactly. The separate K projection
(32 matmuls + biases) disappears; raw x2 chunks serve directly as the
stationary operand of the score matmuls.

Per-core layout: x[b] is already [C, N] channel-major = transposed tokens.
All intermediates stay transposed:
    T1T = M^T X + r [C, N]  (PE: lhsT = M chunks, rhs = X chunks) {fp16}
    V  = X^T @ Wv^T + bv [N, C]  (PE: lhsT = X chunks, rhs = Wv^T) {bf16}
    ST = Xo-block^T @ T1T = logits^T [m, n] blocks (softmax over m)
    E  = exp(ST) {bf16}  (no max subtraction: |logits| < ~40 << 88, fp32-safe)
    U  = accum_m V^T-block @ E -> [C, n] unnormalized out^T {bf16 matmul}
    D  = column sums of E (f32r dacc adds + ones-matmul replicates)
    OT = U / D  [C, N]. No transposes anywhere.

Dtype split (validated by numpy sim + HW; gate 2e-2):
 - S-path (x inputs, M, T1T, score matmuls) in float16: 10 mantissa bits is
   the same precision class as f32r's FP22 truncation (sim: 4.9e-3 vs 4.7e-3)
   but, being non-fp32, gets fast weight load (FWL): per-matmul cost drops
   from ~234ns to ~216ns at 512 free columns. Full bf16 (8 bits) fails the
   gate (sim: 2.6e-2) because exp amplifies q/k rounding. fp16 inputs also
   halve the input DMA bytes (front-latency bound) and SBUF footprint.
 - E must stay bf16: exp values reach e^26 >> fp16's 65504 max.
 - U-path (V tiles, E tiles, U matmuls) in bf16 (same FWL benefit).
 - D accumulates f32r += bf16 on DVE (fp32 internally); PSUM always fp32.

Perf notes (evolved over NTFF traces; 355us -> 290us -> this build):
 - attention uses 1-bank [128,512] score psum tiles (bufs=4 pipeline), one exp
   per 512-half, one dacc add per key block covering the full 1024-wide pair.
 - each pair's normalize/store tail is deferred into the NEXT pair's stream.
 - HAM warmup: ~8 dependency-free matmuls on a zeroed tile bridge the initial
   DMA window so the PE clock gate arms (K=8/8) before real work. DMA data
   only starts flowing ~8.3us in (runtime bootstrap) and drains in issue
   order at the aggregate HBM rate, so input DMAs are full-width transfers
   (>=1KB per-partition lines) ordered strictly by first PE consumption.
 - projections are dissolved into pair 0's key-block stream just-in-time.
 - weights/biases are pre-swizzled on host so every input DMA is a plain 2D
   contiguous transfer; input DMAs are ordered by first consumption.
 - final-pair tail: the denominator matmuls + reciprocals depend only on E
   (not U), so they are emitted right after the last dacc add and overlap the
   remaining U matmuls; the drain holds only the 4 normalize multiplies +
   output DMA. Last dacc add is split per-512-half to release them earlier.
 - reciprocal_approx_fast (18 bits) for the softmax denominators.
"""

import numpy as np

import concourse.bacc as bacc
import concourse.mybir as mybir
import concourse.tile as tile
from concourse.bass_utils import run_bass_kernel_spmd

F32 = mybir.dt.float32
F32R = mybir.dt.float32r
F16 = mybir.dt.float16
BF16 = mybir.dt.bfloat16
AF = mybir.ActivationFunctionType

B, C, H, W = 4, 256, 64, 64
N = H * W            # 4096 tokens
CK = C // 128        # 2 channel chunks of 128
NT = 512             # attention n-tile (query block, free dim)
NP = 1024            # n-tile pair width
N_PAIR = N // NP     # 4
MB = 128             # key/value block (partition block)
N_MB = N // MB       # 32
MB_PER_PIECE = NP // MB   # 8 key blocks per x piece
SKEW = 3             # software-pipeline skew between S and U matmuls

_NC_CACHE = None
LAST_RESULT = None   # BassKernelResults of the most recent kernel() call


def _build():
    nc = bacc.Bacc("TRN2", target_bir_lowering=False, debug=False)

    xa = nc.dram_tensor("xa", [C, N], F16, kind="ExternalInput").ap()
    xb = nc.dram_tensor("xb", [C, N], F16, kind="ExternalInput").ap()
    # host pre-swizzled: [128, CK*C] with (ki, co*128+j) element order
    mt = nc.dram_tensor("mt_l", [128, CK * C], F16, kind="ExternalInput").ap()
    wv = nc.dram_tensor("wv_l", [128, CK * C], F16, kind="ExternalInput").ap()
    rq = nc.dram_tensor("rq_l", [128, CK], F32, kind="ExternalInput").ap()
    bv = nc.dram_tensor("bv_l", [1, C], F32, kind="ExternalInput").ap()
    out = nc.dram_tensor("o", [C, N], F32, kind="ExternalOutput").ap()

    with tile.TileContext(nc) as tc:
        with tc.tile_pool(name="persist", bufs=1) as pp, \
             tc.tile_pool(name="os", bufs=4) as op_, \
             tc.tile_pool(name="s_ps", bufs=4, space="PSUM") as sp, \
             tc.tile_pool(name="u_ps", bufs=1, space="PSUM") as up, \
             tc.tile_pool(name="e_sb", bufs=5) as ep, \
             tc.tile_pool(name="acc", bufs=2) as ap_:
            # ---- HAM warmup (emitted first, zero data deps) -----------
            warm_src = pp.tile([128, NT], F32R, tag="warm_src")
            nc.vector.memset(warm_src[:].bitcast(F32), 0.0)
            warm_ps = sp.tile([128, NT], F32, tag="s")
            N_WARM = 10
            for it in range(N_WARM):
                nc.tensor.matmul(warm_ps[:], warm_src[:, 0:128], warm_src[:],
                                 start=(it == 0), stop=(it == N_WARM - 1))

            # ---- parameters & inputs, in consumption order ------------
            # mt is co-major ([128, co, ki, 128], host-swizzled) so the
            # co=0 half needed by the first projections is one contiguous DMA
            w_r = {"mt": pp.tile([128, CK, CK, 128], F16, name="mt_r",
                                 tag="mt_r"),
                   "wv": pp.tile([128, CK, C], F16, name="wv_r",
                                 tag="wv_r")}
            rq_sb = pp.tile([128, CK], F32, tag="rq_sb")
            bv_r = pp.tile([1, C], F32R, tag="bv_r")
            xa_pieces = {}
            xa_q = {}
            for ki in range(CK):
                for h in range(2):
                    xa_q[(ki, h)] = pp.tile(
                        [128, NT], F16, name=f"xaq_{ki}_{h}", tag=f"xaq_{ki}_{h}")
            for pc in range(1, 4):
                for ki in range(CK):
                    xa_pieces[(ki, pc)] = pp.tile(
                        [128, NP], F16, name=f"xa_{ki}_{pc}", tag=f"xa_{ki}_{pc}")

            def xa_rhs(ki, nt):
                # 512-wide rhs slice of xa for T1T tile nt
                if nt < 2:
                    return xa_q[(ki, nt)][:]
                piece = xa_pieces[(ki, nt // 2)]
                return piece[:, (nt % 2) * NT:((nt % 2) + 1) * NT]

            def xa_mb(ki, mb):
                # 128-wide lhsT slice of xa for V block mb
                if mb < MB_PER_PIECE:
                    t = xa_q[(ki, mb // 4)]
                    return t[:, (mb % 4) * 128:((mb % 4) + 1) * 128]
                piece = xa_pieces[(ki, mb // MB_PER_PIECE)]
                off = (mb % MB_PER_PIECE) * 128
                return piece[:, off:off + 128]

            # xb pieces persist: they are the stationary operand of every
            # score matmul (all 4 pairs sweep all 32 key blocks)
            xb_pieces = {}
            for pc in range(4):
                for ki in range(CK):
                    xb_pieces[(ki, pc)] = pp.tile(
                        [128, NP], F16, name=f"xb_{ki}_{pc}",
                        tag=f"xb_{ki}_{pc}")

            def xb_mb(ki, mb):
                piece = xb_pieces[(ki, mb // MB_PER_PIECE)]
                off = (mb % MB_PER_PIECE) * 128
                return piece[:, off:off + 128]

            # DMA data only starts flowing ~8.3us in (fixed runtime
            # bootstrap) and then drains roughly in issue order at the
            # aggregate HBM rate, so completion time of tensor k ~= 8.3us +
            # cumulative-bytes-before-k / ~250GB/s. Keep per-partition lines
            # >=2KB (full-width transfers) and order strictly by first PE
            # consumption: mt-co0 -> xa_q h0 -> wv -> mt-co1 -> xa_q h1 ->
            # xb piece 0 -> interleaved xa/xb pieces 1-3.
            def dma_mt(co):
                nc.sync.dma_start(
                    w_r["mt"][:, co].rearrange("p k m -> p (k m)"),
                    mt[:, co * C:(co + 1) * C])

            def dma_piece(pieces, src, ki, pc, half=None):
                if half is None:
                    lo, hi = 0, NP
                else:
                    lo, hi = half * NT, (half + 1) * NT
                nc.sync.dma_start(
                    pieces[(ki, pc)][:, lo:hi],
                    src[ki * 128:(ki + 1) * 128,
                        pc * NP + lo:pc * NP + hi])

            dma_mt(0)
            nc.sync.dma_start(rq_sb[:], rq[:])
            nc.sync.dma_start(bv_r[:], bv.bitcast(F32R))
            for ki in range(CK):            # xa_q h=0 (first T1T rhs, V 0-3)
                nc.sync.dma_start(
                    xa_q[(ki, 0)][:],
                    xa[ki * 128:(ki + 1) * 128, 0:NT])
            dma_mt(1)
            for ki in range(CK):            # xa_q h=1 (T1T nt=1, V 4-7)
                nc.sync.dma_start(
                    xa_q[(ki, 1)][:],
                    xa[ki * 128:(ki + 1) * 128, NT:NP])
            for ki in range(CK):            # xb piece 0 blocks 0-3 (S(0..3))
                dma_piece(xb_pieces, xb, ki, 0, 0)
            nc.sync.dma_start(w_r["wv"][:].rearrange("p k m -> p (k m)"),
                              wv)
            for ki in range(CK):            # xb piece 0 blocks 4-7
                dma_piece(xb_pieces, xb, ki, 0, 1)
            for pc in range(1, 4):          # later pieces, full width
                for ki in range(CK):
                    dma_piece(xa_pieces, xa, ki, pc)
                for ki in range(CK):
                    dma_piece(xb_pieces, xb, ki, pc)

            ones_f = pp.tile([128, 128], F32, tag="ones_f")
            nc.vector.memset(ones_f[:], 1.0)
            ones_r = pp.tile([128, 128], F32R, tag="ones_r")
            nc.vector.tensor_copy(ones_r[:], ones_f[:])
            bv_rep = pp.tile([128, CK, 128], F32, tag="bv_rep")

            # per-pair piece tiles for T1T (f32r) / V (bf16)
            qt_p = [pp.tile([128, CK, NP], F16, name=f"qt_{p}", tag=f"qt_{p}")
                    for p in range(N_PAIR)]
            v_p = [pp.tile([128, CK, NP], BF16, name=f"v_{p}", tag=f"v_{p}")
                   for p in range(N_PAIR)]

            # ---- projection emitters ---------------------------------
            def emit_qt(co, nt):
                # T1T' tile: M^T @ xa chunk + r  (f32r)
                ps = sp.tile([128, NT], F32, tag="s")
                half = ps[:]
                for ki in range(CK):
                    nc.tensor.matmul(
                        half, w_r["mt"][:, co, ki],
                        xa_rhs(ki, nt), start=(ki == 0), stop=(ki == CK - 1))
                nc.vector.tensor_scalar_add(
                    qt_p[nt // 2][:, co, (nt % 2) * NT:((nt % 2) + 1) * NT],
                    half, rq_sb[:, co:co + 1])

            def emit_v(mb):
                ps = sp.tile([128, NT], F32, tag="s")
                half = ps[:, 0:C]
                for ki in range(CK):
                    nc.tensor.matmul(
                        half, xa_mb(ki, mb), w_r["wv"][:, ki, :],
                        start=(ki == 0), stop=(ki == CK - 1))
                off = (mb % MB_PER_PIECE) * 128
                nc.vector.tensor_add(
                    v_p[mb // MB_PER_PIECE][:, :, off:off + 128],
                    half.rearrange("p (c j) -> p c j", c=CK), bv_rep[:])

            # ---- attention emitters ----------------------------------
            pair_state = {}

            def attn_begin(pc):
                pair_state[pc] = {
                    "u": [up.tile([128, NP], F32, name=f"u_{pc}_{co}",
                                  tag=f"u{co}") for co in range(CK)],
                    "dacc": ap_.tile([128, NP], F32R, name=f"dacc_{pc}",
                                     tag="dacc"),
                    "e": {},
                }

            def attn_step(pc, step):
                st = pair_state[pc]
                if step < N_MB:
                    mb = step
                    s_h = [sp.tile([128, NT], F32, name=f"s_h{ho}", tag="s")
                           for ho in range(2)]
                    for ki in range(CK):
                        for ho in range(2):
                            nc.tensor.matmul(
                                s_h[ho][:],
                                xb_mb(ki, mb),
                                qt_p[pc][:, ki, ho * NT:(ho + 1) * NT],
                                start=(ki == 0), stop=(ki == CK - 1))
                    e_r = ep.tile([128, NP], BF16, tag="e")
                    for ho in range(2):
                        nc.scalar.activation(
                            e_r[:, ho * NT:(ho + 1) * NT], s_h[ho][:], AF.Exp)
                    st["e"][mb] = e_r
                    # dacc: split the last block per-half so the first
                    # denominator matmul can issue before the second half adds
                    last = (mb == N_MB - 1)
                    if mb == 0:
                        nc.vector.tensor_copy(st["dacc"][:], e_r[:])
                    elif last:
                        for ho in range(2):
                            sl = slice(ho * NT, (ho + 1) * NT)
                            nc.vector.tensor_add(
                                st["dacc"][:, sl], st["dacc"][:, sl],
                                e_r[:, sl])
                    else:
                        nc.vector.tensor_add(st["dacc"][:], st["dacc"][:],
                                             e_r[:])
                if step >= SKEW:
                    mb = step - SKEW
                    e_r = st["e"].pop(mb)
                    vp = v_p[mb // MB_PER_PIECE]
                    off = (mb % MB_PER_PIECE) * 128
                    for co in range(CK):
                        for ho in range(2):
                            nc.tensor.matmul(
                                st["u"][co][:, ho * NT:(ho + 1) * NT],
                                vp[:, co, off:off + 128],
                                e_r[:, ho * NT:(ho + 1) * NT],
                                start=(mb == 0), stop=(mb == N_MB - 1))

            def attn_end_d(pc):
                # denominator matmuls + reciprocals. These depend only on the
                # completed dacc (all E blocks), NOT on the U matmuls -- for
                # the final pair they are emitted right after the last dacc
                # add so they overlap the remaining U matmuls and the drain
                # tail holds only the normalize multiplies.
                st = pair_state[pc]
                dinv = ap_.tile([128, NP], F32, name=f"dinv_{pc}", tag="dinv")
                st["dinv"] = dinv
                for ho in range(2):
                    sl = slice(ho * NT, (ho + 1) * NT)
                    d_ps = sp.tile([128, NT], F32, name=f"d_{ho}", tag="s")
                    nc.tensor.matmul(d_ps[:], ones_r[:], st["dacc"][:, sl],
                                     start=True, stop=True)
                    nc.vector.reciprocal_approx_fast(dinv[:, sl], d_ps[:])

            def attn_end_mul(pc):
                # normalize multiplies ordered to match U completion order
                # (co outer in attn_step), each followed by its store
                st = pair_state.pop(pc)
                dinv = st["dinv"]
                for co in range(CK):
                    for ho in range(2):
                        sl = slice(ho * NT, (ho + 1) * NT)
                        o_sb = op_.tile([128, NT], F32, tag="o_sb")
                        nc.vector.tensor_mul(
                            o_sb[:], st["u"][co][:, sl], dinv[:, sl])
                        nc.sync.dma_start(
                            out[co * 128:(co + 1) * 128,
                                pc * NP + ho * NT:pc * NP + (ho + 1) * NT],
                            o_sb[:])

            # ---- emission schedule -----------------------------------
            # Projection work is distributed just-in-time through pair 0's
            # key-block stream so the PE always has dense 512-free matmul
            # work and the HAM clock gate never re-throttles. Only the
            # piece-0 prerequisites run up front, ordered to match DMA
            # arrival: nt=0 projections (xa_q h0), then nt=1 (mt-co1, xa_q
            # h1). V blocks are deferred into pair 0's steps (first U needs
            # V0 only at step 3, by which time wv has arrived). Warm filler
            # matmuls bridge the gap until xb piece 0 lands so the HAM clock
            # gate never sees an idle window.
            for co in range(CK):
                emit_qt(co, 0)
            # bv replicated to all partitions once (K=1 ones matmul), so the
            # per-block V bias is a DVE add instead of an extra tiny matmul.
            # Emitted after the first T1T tiles: the PE queue is strictly
            # in-order and bv's DMA must not gate the projection matmuls.
            bv_ps = sp.tile([128, NT], F32, tag="s")
            nc.tensor.matmul(bv_ps[:, 0:C], ones_r[0:1, :], bv_r[:],
                             start=True, stop=True)
            nc.vector.tensor_copy(
                bv_rep[:], bv_ps[:, 0:C].rearrange("p (c j) -> p c j", c=CK))
            for co in range(CK):
                emit_qt(co, 1)
            warm2_ps = sp.tile([128, NT], F32, tag="s")
            for it in range(4):
                nc.tensor.matmul(warm2_ps[:], warm_src[:, 0:128],
                                 warm_src[:], start=(it == 0), stop=(it == 3))

            # just-in-time jobs sprinkled through pair 0's steps
            extra = {}

            def add_extra(step, fn):
                extra.setdefault(step, []).append(fn)

            for mb in range(8):             # v blocks 0-7: 2 per early step
                add_extra(mb // 2, lambda mb=mb: emit_v(mb))
            for mb in range(8, N_MB):       # v block 4 steps ahead of its U
                add_extra(mb - 4, lambda mb=mb: emit_v(mb))
            for p in range(1, 4):           # qt pieces 1-3 anywhere in pair 0
                for i, (co, nto) in enumerate(
                        ((0, 0), (0, 1), (1, 0), (1, 1))):
                    add_extra(2 + p * 6 + i,
                              lambda co=co, nt=2 * p + nto: emit_qt(co, nt))

            attn_begin(0)
            for step in range(N_MB + SKEW):
                attn_step(0, step)
                for fn in extra.pop(step, ()):
                    fn()
            # Pair pc-1's dacc is complete during its U-only drain steps, so
            # its whole normalize tail is emitted at the HEAD of pair pc:
            # the d-matmuls enter the PE queue ready-to-run, and the
            # recip/mul chain leads the DVE queue (ahead of pair pc's dacc
            # adds), releasing the U PSUM banks before pair pc's first U
            # matmul needs them.
            last = N_PAIR - 1
            for pc in range(1, N_PAIR):
                attn_begin(pc)
                attn_end_d(pc - 1)
                attn_end_mul(pc - 1)
                for step in range(N_MB + SKEW):
                    attn_step(pc, step)
                    if pc == last and step == N_MB - 1:
                        attn_end_d(last)
            attn_end_mul(last)
    nc.compile()
    return nc


def _get_nc():
    global _NC_CACHE
    if _NC_CACHE is None:
        _NC_CACHE = _build()
    return _NC_CACHE


def _w_layout(w):
    # lhsT chunks, ki-major: w_l[p, ki*C + m] = W.T[ki*128 + p, m]
    wt = np.ascontiguousarray(np.asarray(w, np.float32).T)      # [C_in, C_out]
    return np.ascontiguousarray(
        wt.reshape(CK, 128, C).transpose(1, 0, 2).reshape(128, CK * C))


def _w_layout_co(w):
    # lhsT chunks, co-major: w_l[p, (co*CK + ki)*128 + j] = W.T[ki*128+p,
    # co*128+j] -- the co=0 half is contiguous for a standalone early DMA
    wt = np.asarray(w, np.float32).T                            # [C_in, C_out]
    return np.ascontiguousarray(
        wt.reshape(CK, 128, CK, 128).transpose(1, 2, 0, 3).reshape(128, CK * C))


def kernel(x1, x2, Wq, bq, Wk, bk, Wv, bv):
    global LAST_RESULT
    x1 = np.asarray(x1, dtype=np.float32)
    x2 = np.asarray(x2, dtype=np.float32)
    Wq64 = np.asarray(Wq, np.float64)
    Wk64 = np.asarray(Wk, np.float64)
    # fold Q/K projections: logits = x1^T M x2 + (r . x2_m) + const_n
    M = (Wq64.T @ Wk64).astype(np.float32)          # [C_in1, C_in2]
    r = (Wk64.T @ np.asarray(bq, np.float64)).astype(np.float32)
    shared = {
        "mt_l": _w_layout_co(M.T).astype(np.float16),  # weight for M^T x1
        "wv_l": _w_layout(Wv).astype(np.float16),
        "rq_l": np.ascontiguousarray(r.reshape(CK, 128).T),
        "bv_l": np.asarray(bv, np.float32).reshape(1, C),
    }
    x1h = x1.astype(np.float16)
    x2h = x2.astype(np.float16)
    in_maps = []
    for core in range(8):
        b, s = core % B, core // B
        xs, xo = (x1h, x2h) if s == 0 else (x2h, x1h)
        in_maps.append({
            "xa": np.ascontiguousarray(xs[b].reshape(C, N)),
            "xb": np.ascontiguousarray(xo[b].reshape(C, N)),
            **shared,
        })
    nc = _get_nc()
    res = run_bass_kernel_spmd(nc, in_maps, list(range(8)))
    LAST_RESULT = res
    x1_out = np.stack([res.results[b]["o"].reshape(C, H, W) for b in range(B)])
    x2_out = np.stack([res.results[B + b]["o"].reshape(C, H, W) for b in range(B)])
    return (x1_out, x2_out)
